# revision 28
# baseline (speedup 1.0000x reference)
"""CommutatorConv2d kernel for Trainium2 (Bass/Tile), 8-core data-parallel.

Math: the reference's commutator/anticommutator conv reduces exactly to a
single-channel 3x3 conv on the channel-summed input:

    out[b] = T @ xs[b] @ A + Bm @ xs[b] @ T + bias,   xs = x.sum(axis=1)

where T is the 128x128 tridiagonal-ones matrix and A, Bm are tridiagonal
matrices built from K's column/row sums scaled by (lambda_c +/- lambda_a):
sum_{i,m} XK[...,i,m] = sum_{i,j} patch[i,j]*colsum(K)[j] and
sum_{j,i} KX[...,j,i] = sum_{m,i} patch[m,i]*rowsum(K)[m], so the effective
3x3 kernel is W[i,j] = a[j] + b[i], separable into a row-conv on the vertical
boxsum plus a col-conv on the horizontal boxsum = the two matrix sandwiches.

Precision: x and the constant matrices are cast to bf16 on the host. All
accumulations happen in fp32 PSUM; only element roundings are bf16, giving
~4e-3 relative error against the fp32 reference (the gate is 2e-2) while
halving HBM traffic (the kernel is HBM-bound) and doubling PE/DVE rates.

Layout: each core's shard is sent as [H, B_loc, C, W] (h-major) so every
SBUF partition receives one contiguous >=512B DRAM run per piece.

Schedule: every load is issued up-front across the two HWDGE queues, which
are serviced in lockstep (per-queue position k lands at roughly the same
wall time on both), so each queue's POSITION order is what matters: quad
pieces (PE identity-matmul fold) in early positions so their evac+fold
chain has slack, tree pieces (DVE in-place fold) last, and the final
2-channel sliver lands at the very end where its post-landing chain is just
two adds. uv accumulates the three partial sums (PSUM quad fold + piece-2
tree + piece-3 fold) with back-to-back matmuls against [T | Bm.T]; its PSUM
evacuation is split ACT || DVE; stage 2 multiplies the uv halves by A and
T; the bias-add rides DVE straight out of PSUM; each batch's [H, W] fp32
result stores on its own queue (batch 0 on scalar mid-stream, batch 1 on
sync at the end).
"""

import numpy as np

B, C, H, W = 16, 32, 128, 128
N_CORES = 8
B_LOC = B // N_CORES

_PROGRAM = None
LAST_RESULTS = None


def _build_program():
    import concourse.mybir as mybir
    from concourse import bacc
    from concourse.bass import MemorySpace
    from concourse.tile import TileContext

    f32 = mybir.dt.float32
    bf16 = mybir.dt.bfloat16
    nc = bacc.Bacc(
        "TRN2", target_bir_lowering=False, debug=False, num_devices=N_CORES
    )

    x_dram = nc.dram_tensor("x", (H, B_LOC, C, W), bf16, kind="ExternalInput")
    # fused constants: [T | Bm.T | A | I | bias] as bf16 columns; the last
    # two columns hold each partition's fp32 bias value as raw bits
    cm_dram = nc.dram_tensor("cmat", (H, 4 * W + 2), bf16, kind="ExternalInput")
    out_dram = nc.dram_tensor("out", (H, B_LOC, W), f32, kind="ExternalOutput")

    x_ap = x_dram.ap()
    out_ap = out_dram.ap()

    PIECE = 8  # channels per full piece

    with TileContext(nc) as tc:
        with (
            tc.tile_pool(name="consts", bufs=1) as cpool,
            tc.tile_pool(name="xpool", bufs=2) as xpool,
            tc.tile_pool(name="spool", bufs=2) as spool,
            tc.tile_pool(name="psum", bufs=2, space=MemorySpace.PSUM) as ppool,
        ):
            cm_sb = cpool.tile([H, 4 * W + 2], bf16)
            t_sb = cm_sb[:, 0:W]
            tbm_sb = cm_sb[:, 0 : 2 * W]
            a_sb = cm_sb[:, 2 * W : 3 * W]
            i_sb = cm_sb[:, 3 * W : 4 * W]
            bias_sb = cm_sb[:, 4 * W : 4 * W + 2].bitcast(f32)

            # ---- phase 0: every load issued up-front. Pieces 0/1 are PE
            # quad pieces (8ch), piece 2 a DVE tree piece (8ch), piece 3 a
            # DVE piece loaded as a 6ch run plus a trailing 2ch sliver so
            # the last-landing data needs only two adds before uv.
            tiles = {}
            for b in range(B_LOC):
                for p in range(4):
                    tiles[(b, p)] = xpool.tile(
                        [H, PIECE * W], bf16, tag=f"xq{p}", name=f"xq{b}{p}"
                    )

            def load(eng, b, p, c0, c1, col0):
                xq = tiles[(b, p)]
                eng.dma_start(
                    out=xq[:, col0 * W : (col0 + (c1 - c0)) * W].rearrange(
                        "h (c w) -> h c w", w=W
                    ),
                    in_=x_ap[:, b, c0:c1, :],
                )

            nc.scalar.dma_start(out=cm_sb, in_=cm_dram.ap())
            order = [(0, 0), (0, 1), (0, 2), (0, 3), (1, 0), (1, 1), (1, 2), (1, 3)]
            for idx, (b, p) in enumerate(order):
                eng = nc.sync if idx % 2 == 0 else nc.scalar
                load(eng, b, p, p * PIECE, (p + 1) * PIECE, 0)

            # ---- phase 1, per batch
            for b in range(B_LOC):
                # PE: 4 identity quads fold pieces 0-1 into fp32 PSUM
                cs_psum = ppool.tile([H, 4 * W], f32, tag="csp")
                q = 0
                for p in range(2):
                    for half in range(2):
                        nc.tensor.matmul(
                            cs_psum,
                            i_sb,
                            tiles[(b, p)][:, half * 4 * W : (half + 1) * 4 * W],
                            start=(q == 0),
                            stop=(q == 3),
                        )
                        q += 1
                # ACT evacuates the 4-way partial (rounding to bf16)
                cs = spool.tile([H, 4 * W], bf16, tag="cs")
                nc.scalar.copy(cs, cs_psum)

                # DVE: fold the quad partial and tree-fold piece 2. The 6ch
                # run of piece 3 folds on the otherwise-idle GpSimd engine
                # in parallel; DVE only handles the final 2ch sliver (the
                # one fold left when the stream ends) and its merge.
                nc.vector.tensor_add(
                    cs[:, 0 : 2 * W], cs[:, 0 : 2 * W], cs[:, 2 * W : 4 * W]
                )
                nc.vector.tensor_add(cs[:, 0:W], cs[:, 0:W], cs[:, W : 2 * W])
                p2 = tiles[(b, 2)]
                n = PIECE * W
                while n > W:
                    n //= 2
                    nc.vector.tensor_add(p2[:, :n], p2[:, :n], p2[:, n : 2 * n])
                p3 = tiles[(b, 3)]
                n = PIECE * W
                while n > W:
                    n //= 2
                    nc.vector.tensor_add(p3[:, :n], p3[:, :n], p3[:, n : 2 * n])

                # PE: uv accumulates all three partial sums against [T|Bm.T]
                uv_psum = ppool.tile([H, 2 * W], f32, tag="uvp")
                nc.tensor.matmul(uv_psum, cs[:, 0:W], tbm_sb, start=True, stop=False)
                nc.tensor.matmul(
                    uv_psum, p2[:, 0:W], tbm_sb, start=False, stop=False
                )
                nc.tensor.matmul(
                    uv_psum, p3[:, 0:W], tbm_sb, start=False, stop=True
                )
                uv = spool.tile([H, 2 * W], bf16, tag="uv")
                # split copies so stage-2's first matmul starts after half
                nc.scalar.copy(uv[:, 0:W], uv_psum[:, 0:W])
                nc.scalar.copy(uv[:, W : 2 * W], uv_psum[:, W : 2 * W])

                op = ppool.tile([H, W], f32, tag="op")
                nc.tensor.matmul(op, uv[:, 0:W], a_sb, start=True, stop=False)
                nc.tensor.matmul(op, uv[:, W : 2 * W], t_sb, start=False, stop=True)

                o2b = spool.tile([H, W], f32, tag="o2")
                nc.scalar.add(o2b, op, add=bias_sb)
                eng = nc.scalar if b == 0 else nc.sync
                eng.dma_start(out=out_ap[:, b, :], in_=o2b)

    nc.compile()
    return nc


def _get_program():
    global _PROGRAM
    if _PROGRAM is None:
        _PROGRAM = _build_program()
    return _PROGRAM


def _build_consts(K, bias, lambda_c, lambda_a, np_bf16):
    K = np.asarray(K, np.float32)
    lc = float(np.asarray(lambda_c))
    la = float(np.asarray(lambda_a))
    a = (lc + la) * K.sum(axis=0)  # column sums -> horizontal taps
    b = (la - lc) * K.sum(axis=1)  # row sums -> vertical taps
    eye = np.eye(H, dtype=np.float32)
    up = np.eye(H, k=1, dtype=np.float32)
    dn = np.eye(H, k=-1, dtype=np.float32)
    T = eye + up + dn
    A = a[1] * eye + a[0] * up + a[2] * dn
    Bm = b[1] * eye + b[2] * up + b[0] * dn
    # fused [T | Bm.T | A | I] in bf16, then the fp32 bias bit-packed into
    # two trailing bf16 columns
    cm = np.concatenate([T, Bm.T, A, eye], axis=1).astype(np_bf16)
    bias_col = np.full(
        (H, 1), np.asarray(bias, np.float32).reshape(-1)[0], np.float32
    )
    bias_bits = bias_col.view(np.uint16).view(np_bf16)  # [H, 2] raw halves
    return np.ascontiguousarray(np.concatenate([cm, bias_bits], axis=1))


def kernel(x, K, bias, lambda_c, lambda_a, _trace=False):
    global LAST_RESULTS
    import concourse.mybir as mybir
    from concourse.bass_utils import run_bass_kernel_spmd

    np_bf16 = mybir.dt.np(mybir.dt.bfloat16)
    x = np.asarray(x, np.float32)
    cm = _build_consts(K, bias, lambda_c, lambda_a, np_bf16)
    nc = _get_program()

    in_maps = []
    for core in range(N_CORES):
        shard = x[core * B_LOC : (core + 1) * B_LOC]  # [B_LOC, C, H, W]
        shard_t = np.ascontiguousarray(
            shard.transpose(2, 0, 1, 3).astype(np_bf16)
        )  # [H, B, C, W] bf16
        in_maps.append({"x": shard_t, "cmat": cm})

    res = run_bass_kernel_spmd(
        nc, in_maps, core_ids=list(range(N_CORES)), trace=_trace
    )
    LAST_RESULTS = res
    # per-core outputs are [H, B_LOC, W]; swap back to [B_LOC, H, W]
    out = np.concatenate(
        [r["out"].transpose(1, 0, 2) for r in res.results], axis=0
    )
    return out.reshape(B, 1, H, W).astype(np.float32, copy=False)
